# revision 39
# baseline (speedup 1.0000x reference)
"""Trainium2 Bass kernel: BERT self-attention with hard head-gating.

The reference computes standard multi-head attention, then multiplies the
per-(batch, head) attention probabilities by a hard gate (logits >= 0)
produced by a tiny MLP over the mean-pooled hidden states.  A gated-off
head contributes exactly zero to the output, so the host evaluates the
gate MLP (a few thousand flops) and only schedules the ON heads on the
device, sharded across the 8 NeuronCores (data-parallel over batch,
head-parallel within batch, per the sharding hint).

Device kernel per core (SPMD, per-core data differs):
  - bf16 data path (tolerance is 2e-2; lands ~7e-3): x arrives as the
    pre-swizzled SBUF image so every chunk is one large contiguous DMA,
    split across both HWDGE queues.
  - the PE HAM activity monitor keeps the PE clock at 1.2 GHz until it
    has seen sustained full-array activity (the baseline ramped to
    2.4 GHz only at t=21.5us).  A stream of dummy full-array matmuls on
    a zeroed scratch tile starts at t~0.5us, so the HAM has ramped by
    the time the first real projection runs.
  - phase order is restructured so the attention loop starts as early
    as possible: only K-ch0 + Q-ch0 + Q-ch1 projections run before the
    first scores matmul (they are all that (qg0, kt0) needs); the
    remaining K/Q chunks and all of V become PE quanta that fill the
    slack inside the attention loop (the loop is PE-paced: in-loop PE
    ~81us vs ACT exp stream ~66us).
  - V^T is produced DIRECTLY by x-stationary matmuls (stationary =
    [128 dims x 128 positions] x-tile, moving = Wv tile), accumulating
    [128 pos, 128 packed dims] in PSUM over the 8 D-tiles.  This
    replaces the V projection + 16 PE transposes and their PSUM pools.
  - all attention matmuls are FULL-ARRAY (K=128, M=128): Q is stored
    twice, zero-padded on the other slot's 64 partitions, so each
    slot's scores matmul contracts over all 128 partitions against the
    SHARED packed-K stationary (all 4 scores matmuls of a k-tile reuse
    one LDWEIGHTS).
  - exp(0.125*scores + mask) is fused on ScalarE (PSUM -> SBUF bf16),
    the mask entering as the per-partition activation bias; the ones
    column of V+ accumulates the softmax denominator as psum row 64.
  - ctx matmuls are deferred (exp outputs buffer in SBUF) until the
    V quanta finish and the projection psum banks can be handed to the
    ctx accumulators.  PSUM: pp(2)+vt(2)+scs(4) -> 8 banks during the
    overlap, then scs(4)+acc(4).
  - the unnormalized [ctx^T; rowsum] block is copied to SBUF (VectorE)
    and DMA'd out; the host divides by the rowsum row, adds bv (exact:
    ctx/sum + bv == sum(e*(v+bv))/sum(e)), and transposes while
    scattering into the full output.
  - a post-build pass drops LDWEIGHTS that reload what the PE already
    holds (the tile legalizer pre-splits bf16 matmuls but never
    dedupes, and walrus's ldw-opt rejects pre-split LDWEIGHTS).
"""

import math
import os
import sys
import types

os.environ.setdefault("JAX_PLATFORMS", "axon")

import numpy as np

B, S, D, H, HD = 2, 2048, 1024, 16, 64
P = 128
FD = 512          # fp32 psum bank / matmul moving-operand chunk
QG = 1024         # attention q-group size (psum bank budget)
NDT = D // P      # 8 D-tiles
NCH = S // FD     # 4 projection rhs chunks
NKT = S // P      # 16 k-tiles
NQG = S // QG     # 2
CW = NDT * FD     # x_sb columns per projection chunk
BN_EPS = 1e-12
NWARM = 10        # HAM-ramp dummy matmuls before the first projection

_PROG_CACHE = {}
LAST_EXEC_TIME_NS = None


def _install_ntff_hook():
    """This image's antenv package lacks axon_hooks; recreate it so
    run_bass_kernel_spmd(trace=True) can reach the NTFF profiler."""
    if "antenv.axon_hooks" in sys.modules:
        return
    if "/root/.axon_site" not in sys.path:
        sys.path.insert(0, "/root/.axon_site")
    try:
        from trn_agent_boot.trn_boot import _ntff_profile_via_ctypes
        hook = _ntff_profile_via_ctypes("/opt/axon/libaxon_pjrt.so")
    except Exception:
        hook = None
    m = types.ModuleType("antenv.axon_hooks")
    m.get_axon_ntff_profile_hook = lambda: hook
    m.set_axon_ntff_profile_hook = lambda h: None
    sys.modules["antenv.axon_hooks"] = m


def _dedupe_ldweights(nc, mybir):
    """The tile legalizer pre-splits 2-byte matmuls into LDWEIGHTS+MATMUL
    but emits one LDWEIGHTS per matmul even when consecutive matmuls share
    the stationary operand (and walrus's ldw-opt pass, which would fold
    them, rejects pre-split LDWEIGHTS).  Drop an LDWEIGHTS that reloads
    exactly what the PE already holds; a transpose matmul self-loads its
    identity, invalidating the tracked state."""
    for bb in nc.main_func.blocks:
        new = []
        last = None
        for ins in bb.instructions:
            if isinstance(ins, mybir.InstLdweights):
                a = ins.ins[0]
                sig = (a.memref, a.offset, tuple(map(tuple, a.ap)), a.dtype)
                si = ins.sync_info
                clean = si is None or (not si.on_wait and not si.on_update)
                if clean and sig == last:
                    continue
                last = sig
            elif isinstance(ins, mybir.InstMatmult):
                if getattr(ins, "is_transpose", False):
                    last = None
            new.append(ins)
        bb.instructions = new


def _split_sync_waits(nc, mybir):
    """This walrus build rejects instructions carrying more than one
    sync-wait command: hoist extra waits onto EventSemaphore
    instructions inserted just before (same engine stream, so the
    combined wait semantics are identical)."""
    for bb in nc.main_func.blocks:
        new = []
        for ins in bb.instructions:
            si = ins.sync_info
            if si is not None and si.on_wait and len(si.on_wait) > 1:
                waits = list(si.on_wait)
                for w in waits[:-1]:
                    new.append(mybir.InstEventSemaphore(
                        name=f"EVW-{nc.next_id()}",
                        engine=ins.engine,
                        ins=[], outs=[],
                        sync_info=mybir.SyncInfo(on_wait=[w], on_update=[]),
                    ))
                ins.sync_info = mybir.SyncInfo(
                    on_wait=[waits[-1]], on_update=list(si.on_update)
                )
            new.append(ins)
        bb.instructions = new


def _build(npair):
    import concourse.bass as bass
    import concourse.mybir as mybir
    import concourse.tile as tile

    f32 = mybir.dt.float32
    bf16 = mybir.dt.bfloat16
    ts = bass.ts
    _TC = tile.TileContext

    G = 3 * npair
    ns = 2 * npair
    nc = bass.Bass(num_devices=8)
    # xt arrives pre-swizzled by the host into the exact SBUF image
    # [P, NCH*CW] (chunk-major, 8KB contiguous per partition-row per
    # chunk) so each chunk is one large, descriptor-efficient DMA.
    xt = nc.dram_tensor("xt", [P, NCH * CW], bf16, kind="ExternalInput")
    wpk = nc.dram_tensor("wpk", [P, G * NDT * P], bf16, kind="ExternalInput")
    bpk = nc.dram_tensor("bpk", [P, G], f32, kind="ExternalInput")
    mk = nc.dram_tensor("mk", [P, NKT], f32, kind="ExternalInput")
    one = nc.dram_tensor("one", [P, NKT], bf16, kind="ExternalInput")
    # bf16 output: the host divides ctx^T by the rowsum anyway, so the
    # ~0.4% bf16 rounding of numerator and denominator costs ~2e-3 of
    # relative error (measured total stays under half the 2e-2 gate) and
    # halves the tail output-DMA bytes.
    out = nc.dram_tensor("out", [ns, 65, S], bf16, kind="ExternalOutput")
    fence = nc.dram_tensor("fence", [2, 4], bf16, kind="ExternalOutput")

    Exp = mybir.ActivationFunctionType.Exp

    with _TC(nc) as tc, \
         tc.tile_pool(name="const", bufs=1) as cpool, \
         tc.tile_pool(name="xtp", bufs=1) as xpool, \
         tc.tile_pool(name="qkv", bufs=npair) as qkvpool, \
         tc.tile_pool(name="vp", bufs=2) as vpool, \
         tc.tile_pool(name="ep", bufs=44) as epool, \
         tc.tile_pool(name="cup", bufs=4) as cupool:

        # Preload the ACT exp table while input DMAs run.
        warm = cpool.tile([P, 1], f32, name="warm", tag="warm")
        nc.vector.memset(warm[:], 0.0)
        warm2 = cpool.tile([P, 1], f32, name="warm2", tag="warm2")
        nc.scalar.activation(warm2[:], warm[:], Exp, bias=warm[:, 0:1])

        # HAM-ramp spam: dense full-array matmuls on a zeroed scratch
        # tile, no input dependencies, so the PE activity monitor lifts
        # the 1.2 GHz throttle during the DMA dead time instead of 10us
        # into the real projections.  The psum results are never read.
        wu_sb = cpool.tile([P, FD], bf16, name="wu", tag="wu")
        nc.vector.memset(wu_sb[:], 0.0)
        def mk_spam(pool):
            def spam(cols=FD):
                wps = pool.tile([P, FD], f32, name="wps", tag="wps")
                nc.tensor.matmul(wps[:, 0:cols], wu_sb[:, 0:P],
                                 wu_sb[:, 0:cols], start=True, stop=True)
            return spam

        wu_ctx = tc.tile_pool(name="wup", bufs=2, space="PSUM")
        spam0 = mk_spam(wu_ctx.__enter__())
        for _ in range(NWARM):
            spam0()
        wu_ctx.__exit__(None, None, None)

        # wpk is laid out K|Q|V-major by the host (all K groups first),
        # so the K-projection weights can be a small leading DMA and the
        # first matmul starts as early as possible.  x chunk quarters
        # alternate between the two HWDGE queues (sync/scalar); each DMA
        # op lands on its own SDMA engine set, so concurrency across ops
        # is what buys bandwidth.  ch0/ch1 are prioritized: the attention
        # loop needs only K-ch0 + Q-ch0/ch1 to start.
        WG = NDT * P                       # w columns per (type, pair) group
        w_sb = cpool.tile([P, G * WG], bf16, name="w", tag="w")
        x_sb = xpool.tile([P, NCH * CW], bf16, name="x", tag="x")
        QT4 = CW // 4
        # Phase 1: only what the attention loop needs to START (K+Q
        # weights, x-ch0, x-ch1).  The HWDGE queues run several ops
        # concurrently sharing bandwidth, so without this fence every
        # chunk completes near the END of the whole input stream
        # (~23us); narrowing the early queue gets ch0/ch1 in by ~14us.
        nc.sync.dma_start(w_sb[:, 0:npair * WG], wpk[:, 0:npair * WG])       # K
        for ch in range(2):
            for qq in range(4):
                eng = nc.scalar if qq % 2 == 0 else nc.sync
                eng.dma_start(
                    x_sb[:, ch * CW + qq * QT4: ch * CW + (qq + 1) * QT4],
                    xt[:, ch * CW + qq * QT4: ch * CW + (qq + 1) * QT4])
            if ch == 0:                                                       # Q
                nc.scalar.dma_start(
                    w_sb[:, npair * WG:2 * npair * WG],
                    wpk[:, npair * WG:2 * npair * WG])
        # Fences: a tiny readback of the tail of each queue's last ch1
        # piece; the phase-2 ops behind it wait for its semaphore, so
        # they stay out of the queue until ch1 has landed.  The fence
        # target (a corner of `out`) is overwritten by the real output.
        nc.sync.dma_start(fence[0:1, 0:1],
                          x_sb[0:1, 2 * CW - 4:2 * CW - 3])
        nc.scalar.dma_start(fence[1:2, 0:1],
                            x_sb[0:1, 2 * CW - QT4 - 1:2 * CW - QT4])
        for ch in range(2, NCH):
            dst = x_sb[:, ch * CW:(ch + 1) * CW]
            src = xt[:, ch * CW:(ch + 1) * CW]
            nc.sync.dma_start(dst[:, 0:CW // 2], src[:, 0:CW // 2])
            nc.scalar.dma_start(dst[:, CW // 2:CW], src[:, CW // 2:CW])
        nc.sync.dma_start(                                                    # V
            w_sb[:, 2 * npair * WG:3 * npair * WG],
            wpk[:, 2 * npair * WG:3 * npair * WG])
        b_sb = cpool.tile([P, G], f32, name="b", tag="b")
        nc.gpsimd.dma_start(b_sb[:], bpk[:, :])
        m_sb = cpool.tile([P, NKT], f32, name="m", tag="m")
        nc.gpsimd.dma_start(m_sb[:], mk[:, :])
        on_sb = cpool.tile([P, NKT], bf16, name="on", tag="on")
        nc.gpsimd.dma_start(on_sb[:], one[:, :])

        for p_ in range(npair):
            # Attention matmuls are deliberately FULL-ARRAY (K=128, M=128):
            # partial-array matmuls (K=64 scores / M=65 ctx) never register
            # as "busy" with the PE HAM activity monitor.  Q is stored
            # twice, zero-padded on the other slot's 64 partitions, so each
            # slot's scores matmul contracts over all 128 partitions against
            # the SHARED packed K stationary.
            kt_sb = qkvpool.tile([P, S], bf16, name="qkvK", tag="qkvK")
            qtz = [qkvpool.tile([P, S], bf16, name=f"qtz{hs}", tag=f"qtz{hs}")
                   for hs in range(2)]
            nc.vector.memset(qtz[0][HD:P, :], 0.0)
            nc.vector.memset(qtz[1][0:HD, :], 0.0)
            vps = []
            for hs in range(2):
                vp = vpool.tile([P, NKT * P], bf16, name="vp", tag="vp")
                nc.vector.memset(vp[:], 0.0)
                nc.vector.tensor_copy(
                    vp[:].rearrange("p (t c) -> p t c", c=P)[:, :, 64:65],
                    on_sb[:, 0:NKT].rearrange("p (t c) -> p t c", c=1),
                )
                vps.append(vp)

            gK = 0 * npair + p_
            gQ = 1 * npair + p_
            gV = 2 * npair + p_

            # PSUM budget: pp(2) + scs(2x2) = 6 banks while the deferred
            # projections overlap attention; pp closes once the last
            # projection quantum ran, freeing banks for the 4-bank ctx
            # accumulators (scs 4 + acc 4 = 8).
            ps_ctx = tc.tile_pool(name="ps", bufs=2, space="PSUM")
            pp_ctx = tc.tile_pool(name="pp", bufs=2, space="PSUM")
            pspool = ps_ctx.__enter__()
            pppool = pp_ctx.__enter__()
            vt_sb = qkvpool.tile([P, S], bf16, name="vtsb", tag="vtsb")
            tt_pool = tc.tile_pool(name="ttp", bufs=2)
            ttpool = tt_pool.__enter__()

            def proj_mms(g, ps, ch, d0, d1):
                for dt in range(d0, d1):
                    nc.tensor.matmul(
                        ps[:],
                        w_sb[:, (g * NDT + dt) * P:(g * NDT + dt + 1) * P],
                        x_sb[:, ch * CW + dt * FD: ch * CW + (dt + 1) * FD],
                        start=(dt == 0),
                        stop=(dt == NDT - 1),
                    )

            def k_finish(ch, ps):
                nc.vector.tensor_scalar_add(
                    kt_sb[:, ch * FD:(ch + 1) * FD], ps[:], b_sb[:, gK:gK + 1])

            def q_finish(ch, ps):
                nc.vector.tensor_scalar_add(
                    qtz[0][0:HD, ch * FD:(ch + 1) * FD], ps[0:HD, :],
                    b_sb[0:HD, gQ:gQ + 1])
                nc.vector.tensor_scalar_add(
                    qtz[1][HD:P, ch * FD:(ch + 1) * FD], ps[HD:P, :],
                    b_sb[HD:P, gQ:gQ + 1])

            # Minimal pre-loop projections: exactly what (qg0, kt0..3)
            # needs -- K-ch0 and Q-ch0/ch1.  The x quarters trickle in
            # ~2us apart while a projection piece takes ~0.9us, so small
            # spam matmuls are interleaved between the dt-pair pieces:
            # an idle PE of even ~1us makes the HAM re-throttle the
            # clock to 1.2 GHz for several us (observed), which is far
            # more expensive than the spam.
            wu2_ctx = tc.tile_pool(name="wup2", bufs=2, space="PSUM")
            spam = mk_spam(wu2_ctx.__enter__())
            for g, ch, fin in ((gK, 0, k_finish), (gQ, 0, q_finish),
                               (gQ, 1, q_finish)):
                ps = pppool.tile([P, FD], f32, name="pp", tag="pp")
                for dp in range(4):
                    proj_mms(g, ps, ch, dp * 2, dp * 2 + 2)
                    spam(256)
                fin(ch, ps)
            wu2_ctx.__exit__(None, None, None)

            def v_finish(ch, ps):
                # V chunk PSUM -> bf16 SBUF; the transposes that carve it
                # into V+ tiles are deferred until the x DMA has drained
                # (they share the HWDGE queues with the x input).
                nc.vector.tensor_copy(vt_sb[:, ch * FD:(ch + 1) * FD], ps[:])

            def t_tile(t):
                # One k-tile of V through the DMA xbar transpose engine
                # (zero PE cost) into a staging tile; two DVE copies split
                # the halves into the V+ tiles (col 64 of each V+ tile is
                # the preset ones column).
                def go():
                    vtt = ttpool.tile([P, P], bf16, name="vtt", tag="vtt")
                    eng = nc.sync if t % 2 == 0 else nc.scalar
                    eng.dma_start_transpose(vtt[:], vt_sb[:, ts(t, P)])
                    nc.vector.tensor_copy(
                        vps[0][:, t * P: t * P + HD], vtt[:, 0:HD])
                    nc.vector.tensor_copy(
                        vps[1][:, t * P: t * P + HD], vtt[:, HD:P])
                return go

            # Deferred projections become a queue of small PE quanta
            # (cost in us, earliest-step gate, must-pop flag) that fill
            # the PE's slack inside the attention loop below.  The
            # earliest-step gates respect x-chunk DMA arrival: the PE
            # queue executes in order, so a projection emitted before its
            # chunk has landed blocks every later PE instruction behind
            # it.  A must-pop item is emitted at its gate step even if
            # the budget ran out -- the next step's scores depend on it,
            # and emitting it any later would deadlock the PE queue.
            pstate = {}

            def p_mm(g, ch, d0, d1, fin):
                def go():
                    if d0 == 0:
                        pstate[(g, ch)] = pppool.tile(
                            [P, FD], f32, name="pp", tag="pp")
                    proj_mms(g, pstate[(g, ch)], ch, d0, d1)
                    if d1 == NDT:
                        fin(ch, pstate[(g, ch)])
                return go

            vwork = []
            for (g, ch, fin), mi, must in (
                ((gK, 1, k_finish), 0, True),   # before scores(kt4) @ 3
                ((gV, 0, v_finish), 1, False),  # x-ch0 resident
                ((gK, 2, k_finish), 3, True),   # before scores(kt8) @ 7
                ((gV, 1, v_finish), 4, False),
                ((gK, 3, k_finish), 5, True),   # before scores(kt12) @ 11
                ((gV, 2, v_finish), 6, False),
                ((gQ, 2, q_finish), 7, True),   # before scores(qg1) @ 15
                ((gQ, 3, q_finish), 8, True),
                ((gV, 3, v_finish), 9, False),
            ):
                vwork.append((p_mm(g, ch, 0, 4, fin), 0.9, mi, must))
                vwork.append((p_mm(g, ch, 4, NDT, fin), 0.9, mi, must))
            # V+ transposes: gated past the x DMA (done ~step 2-3 of the
            # loop) and their source chunk's projection.
            for t in range(NKT):
                gate = max(11, {0: 2, 1: 5, 2: 7, 3: 10}[t // 4] + 1)
                vwork.append((t_tile(t), 0.35, gate, False))
            vwork.sort(key=lambda w: w[2])

            def issue_scores(qg, kt, h2s):
                scs = [pspool.tile([P, QG], f32, name="ps", tag="ps")
                       for _ in range(2)]
                for hs in range(2):
                    for h2 in h2s:
                        nc.tensor.matmul(
                            scs[hs][:, h2 * FD:(h2 + 1) * FD],
                            kt_sb[:, ts(kt, P)],
                            qtz[hs][:, qg * QG + h2 * FD: qg * QG + (h2 + 1) * FD],
                            start=True, stop=True,
                        )
                return scs

            # ---- attention loop, software-pipelined --------------------
            # Step granularity is (qg, kt, h2-set).  The first four
            # k-tiles of qg0 are split into 512-column halves so the exp
            # stream starts after only x-ch0 + K-ch0 + Q-ch0 (h2=1 needs
            # Q-ch1, whose x chunk is still in flight); the last step is
            # split so the final ctx/out work overlaps the last exps.
            # ctx matmuls are deferred (exp outputs buffer in SBUF) until
            # the projection/V quanta finish and the psum banks can be
            # handed to the ctx accumulators.
            acc_ctx = [None]
            accpool_ref = [None]
            ctx_backlog = []          # (qg, kt, hs, h2, e-tile)
            ctx_accs = [None]
            cur_qg = [None]
            done15 = {}

            def open_acc_pool():
                pp_ctx.__exit__(None, None, None)
                acc_ctx[0] = tc.tile_pool(name="accp", bufs=2, space="PSUM")
                accpool_ref[0] = acc_ctx[0].__enter__()

            def drain_ctx(max_entries):
                done = 0
                while ctx_backlog and done < max_entries:
                    qg, kt, hs, h2, e = ctx_backlog.pop(0)
                    if cur_qg[0] != qg:
                        cur_qg[0] = qg
                        ctx_accs[0] = [
                            accpool_ref[0].tile([P, QG], f32, name="acc", tag="acc")
                            for _ in range(2)]
                    accs = ctx_accs[0]
                    nc.tensor.matmul(
                        accs[hs][:, h2 * FD:(h2 + 1) * FD],
                        vps[hs][:, kt * P:(kt + 1) * P],
                        e[:, h2 * FD:(h2 + 1) * FD],
                        start=(kt == 0),
                        stop=(kt == NKT - 1),
                    )
                    if kt == NKT - 1:
                        # bounce this h2-half of [ctx^T; rowsum] PSUM ->
                        # SBUF (hs0 on VectorE, hs1 on ScalarE -- idle
                        # after its last exp) and DMA it out split across
                        # both HWDGE queues, so the final output transfer
                        # overlaps the remaining halves' exps and ctx.
                        s_idx = p_ * 2 + hs
                        cu = cupool.tile([65, FD], bf16, name="cu", tag="cu")
                        cols = slice(h2 * FD, (h2 + 1) * FD)
                        nc.vector.tensor_copy(cu[:], accs[hs][0:65, cols])
                        o0 = qg * QG + h2 * FD
                        hf = FD // 2
                        nc.sync.dma_start(
                            out[s_idx][:, o0:o0 + hf], cu[:, 0:hf])
                        nc.scalar.dma_start(
                            out[s_idx][:, o0 + hf:o0 + FD], cu[:, hf:FD])
                    done += 1

            steps = []
            for kt in range(NKT):
                steps.append((0, kt, (0, 1)))
            for kt in range(NKT - 1):
                steps.append((1, kt, (0, 1)))
            steps.append((1, NKT - 1, (0,)))
            steps.append((1, NKT - 1, (1,)))

            cur = issue_scores(*steps[0])
            for i, (qg, kt, h2s) in enumerate(steps):
                c0, c1 = h2s[0] * FD, (h2s[-1] + 1) * FD
                es2 = []
                for hs in range(2):
                    e = epool.tile([P, QG], bf16, name="e", tag="e")
                    nc.scalar.activation(
                        e[:, c0:c1], cur[hs][:, c0:c1], Exp,
                        bias=m_sb[:, kt:kt + 1], scale=0.125,
                    )
                    es2.append(e)
                # next step's scores go on the PE queue FIRST so the scs
                # psum buffer refills the moment its exp frees it, keeping
                # ScalarE back-to-back.
                nxt = issue_scores(*steps[i + 1]) if i + 1 < len(steps) else None
                for hs in range(2):
                    for h2 in h2s:
                        ctx_backlog.append((qg, kt, hs, h2, es2[hs]))
                if vwork:
                    budget = 1.2
                    while vwork and vwork[0][2] <= i and \
                            (budget > 0 or vwork[0][3]):
                        go, cost, _mi, _must = vwork.pop(0)
                        go()
                        budget -= cost
                else:
                    if acc_ctx[0] is None:
                        open_acc_pool()
                    drain_ctx(6 if len(ctx_backlog) > 8 else 2)
                cur = nxt
            if acc_ctx[0] is None:
                open_acc_pool()
            drain_ctx(len(ctx_backlog))
            acc_ctx[0].__exit__(None, None, None)
            ps_ctx.__exit__(None, None, None)
            tt_pool.__exit__(None, None, None)
    _dedupe_ldweights(nc, mybir)
    _split_sync_waits(nc, mybir)
    return nc


def _np_gates(inputs):
    hs = inputs["hidden_states"].astype(np.float64)
    pooled = hs.mean(axis=1)
    h = pooled @ inputs["pW1"].astype(np.float64) + inputs["pb1"].astype(np.float64)
    h = (h - inputs["bn_mean"].astype(np.float64)) \
        / np.sqrt(inputs["bn_var"].astype(np.float64) + BN_EPS) \
        * inputs["bn_gamma"].astype(np.float64) + inputs["bn_beta"].astype(np.float64)
    h = np.maximum(h, 0.0)
    logits = h @ inputs["pW2"].astype(np.float64) + inputs["pb2"].astype(np.float64)
    return logits >= 0.0


def kernel(**inputs):
    global LAST_EXEC_TIME_NS
    import ml_dtypes
    bf = ml_dtypes.bfloat16

    inputs = {k: np.asarray(v) for k, v in inputs.items()}
    out_full = np.zeros((B, S, D), np.float32)

    gate = _np_gates(inputs)                       # [B, H] bool
    on = [[h for h in range(H) if gate[b, h]] for b in range(B)]
    n0, n1 = len(on[0]), len(on[1])
    if n0 + n1 == 0:
        return out_full

    # Split the 8 cores between the two batches to minimize the max
    # number of head-slots any core has to process.
    best = None
    for k0 in range(9):
        k1 = 8 - k0
        if (n0 > 0 and k0 == 0) or (n1 > 0 and k1 == 0):
            continue
        ns_req = max(
            math.ceil(n0 / k0) if n0 else 0,
            math.ceil(n1 / k1) if n1 else 0,
        )
        if best is None or ns_req < best[0]:
            best = (ns_req, k0)
    ns_req, k0 = best
    k1 = 8 - k0
    npair = (ns_req + 1) // 2
    ns = 2 * npair

    # head-slot assignment per core: (b, h, is_real)
    core_batch = [0 if c < k0 else 1 for c in range(8)]
    core_slots = []
    for c in range(8):
        b = core_batch[c]
        if b == 0:
            mine = on[0][c::k0] if k0 else []
        else:
            mine = on[1][(c - k0)::k1] if k1 else []
        slots = [(b, h, True) for h in mine]
        pad_h = mine[0] if mine else (on[b][0] if on[b] else 0)
        while len(slots) < ns:
            slots.append((b, pad_h, False))
        core_slots.append(slots)

    # per-batch staged arrays; x is pre-swizzled into the SBUF image
    # [P, NCH*CW]: row p, col ch*CW + dt*FD + j  <-  x^T[dt*P + p, ch*FD + j]
    xtb = []
    for b in range(B):
        xT = inputs["hidden_states"][b].T.astype(np.float32).astype(bf)  # [D, S]
        img = (xT.reshape(NDT, P, NCH, FD)      # (dt, p, ch, j)
               .transpose(1, 2, 0, 3)           # (p, ch, dt, j)
               .reshape(P, NCH * CW))
        xtb.append(np.ascontiguousarray(img))
    mkb = [np.ascontiguousarray(
        inputs["attention_mask"][b, 0, 0, :].astype(np.float32)
        .reshape(NKT, P).T) for b in range(B)]
    ones16 = np.ones((P, NKT), bf)

    Ws = (inputs["Wq"].astype(np.float32), inputs["Wk"].astype(np.float32),
          inputs["Wv"].astype(np.float32))
    bs = (inputs["bq"].astype(np.float32), inputs["bk"].astype(np.float32),
          inputs["bv"].astype(np.float32))

    G = 3 * npair
    in_maps = []
    for c in range(8):
        slots = core_slots[c]
        wgs, bgs = [], []
        # group order is type-major (all K pairs, then Q, then V) so the
        # K weights can be the first, small leading DMA on-device.
        for Wsrc, bsrc in ((Ws[1], bs[1]), (Ws[0], bs[0]), (Ws[2], bs[2])):
            for p_ in range(npair):
                h0 = slots[2 * p_][1]
                h1 = slots[2 * p_ + 1][1]
                wgs.append(np.concatenate(
                    [Wsrc[:, h0 * HD:(h0 + 1) * HD],
                     Wsrc[:, h1 * HD:(h1 + 1) * HD]], axis=1))
                bgs.append(np.concatenate(
                    [bsrc[h0 * HD:(h0 + 1) * HD],
                     bsrc[h1 * HD:(h1 + 1) * HD]]))
        wpk = (np.stack(wgs).reshape(G, NDT, P, P)
               .transpose(2, 0, 1, 3).reshape(P, G * NDT * P))
        bpk = np.stack(bgs, axis=1)
        b = core_batch[c]
        in_maps.append({
            "xt": xtb[b],
            "wpk": np.ascontiguousarray(wpk.astype(bf)),
            "bpk": np.ascontiguousarray(bpk),
            "mk": mkb[b],
            "one": ones16,
        })

    trace = os.environ.get("BASS_KERNEL_TRACE") == "1"
    if trace:
        _install_ntff_hook()

    # NOTE: --enable-ldw-opt stays false: the tile legalizer pre-splits
    # bf16 matmuls into LDWEIGHTS+MATMUL, which that walrus pass rejects.
    nc = _PROG_CACHE.get(npair)
    if nc is None:
        nc = _build(npair)
        _PROG_CACHE[npair] = nc

    from concourse.bass_utils import run_bass_kernel_spmd
    res = run_bass_kernel_spmd(
        nc, in_maps, core_ids=list(range(8)), trace=trace)
    LAST_EXEC_TIME_NS = res.exec_time_ns

    bv = inputs["bv"].astype(np.float32)
    for c in range(8):
        co = res.results[c]["out"]            # [ns, 65, S] bf16
        for si, (b, h, real) in enumerate(core_slots[c]):
            if real:
                blk = np.asarray(co[si]).astype(np.float32)
                out_full[b][:, h * HD:(h + 1) * HD] = \
                    (blk[0:64] / blk[64:65]).T + bv[h * HD:(h + 1) * HD][None, :]
    return out_full


# revision 40
# speedup vs baseline: 1.0133x; 1.0133x over previous
"""Trainium2 Bass kernel: BERT self-attention with hard head-gating.

The reference computes standard multi-head attention, then multiplies the
per-(batch, head) attention probabilities by a hard gate (logits >= 0)
produced by a tiny MLP over the mean-pooled hidden states.  A gated-off
head contributes exactly zero to the output, so the host evaluates the
gate MLP (a few thousand flops) and only schedules the ON heads on the
device, sharded across the 8 NeuronCores (data-parallel over batch,
head-parallel within batch, per the sharding hint).

Device kernel per core (SPMD, per-core data differs):
  - bf16 data path (tolerance is 2e-2; lands ~7e-3): x arrives as the
    pre-swizzled SBUF image so every chunk is one large contiguous DMA,
    split across both HWDGE queues.
  - the PE HAM activity monitor keeps the PE clock at 1.2 GHz until it
    has seen sustained full-array activity (the baseline ramped to
    2.4 GHz only at t=21.5us).  A stream of dummy full-array matmuls on
    a zeroed scratch tile starts at t~0.5us, so the HAM has ramped by
    the time the first real projection runs.
  - phase order is restructured so the attention loop starts as early
    as possible: only K-ch0 + Q-ch0 + Q-ch1 projections run before the
    first scores matmul (they are all that (qg0, kt0) needs); the
    remaining K/Q chunks and all of V become PE quanta that fill the
    slack inside the attention loop (the loop is PE-paced: in-loop PE
    ~81us vs ACT exp stream ~66us).
  - V^T is produced DIRECTLY by x-stationary matmuls (stationary =
    [128 dims x 128 positions] x-tile, moving = Wv tile), accumulating
    [128 pos, 128 packed dims] in PSUM over the 8 D-tiles.  This
    replaces the V projection + 16 PE transposes and their PSUM pools.
  - all attention matmuls are FULL-ARRAY (K=128, M=128): Q is stored
    twice, zero-padded on the other slot's 64 partitions, so each
    slot's scores matmul contracts over all 128 partitions against the
    SHARED packed-K stationary (all 4 scores matmuls of a k-tile reuse
    one LDWEIGHTS).
  - exp(0.125*scores + mask) is fused on ScalarE (PSUM -> SBUF bf16),
    the mask entering as the per-partition activation bias; the ones
    column of V+ accumulates the softmax denominator as psum row 64.
  - ctx matmuls are deferred (exp outputs buffer in SBUF) until the
    V quanta finish and the projection psum banks can be handed to the
    ctx accumulators.  PSUM: pp(2)+vt(2)+scs(4) -> 8 banks during the
    overlap, then scs(4)+acc(4).
  - the unnormalized [ctx^T; rowsum] block is copied to SBUF (VectorE)
    and DMA'd out; the host divides by the rowsum row, adds bv (exact:
    ctx/sum + bv == sum(e*(v+bv))/sum(e)), and transposes while
    scattering into the full output.
  - a post-build pass drops LDWEIGHTS that reload what the PE already
    holds (the tile legalizer pre-splits bf16 matmuls but never
    dedupes, and walrus's ldw-opt rejects pre-split LDWEIGHTS).
"""

import math
import os
import sys
import types

os.environ.setdefault("JAX_PLATFORMS", "axon")

import numpy as np

B, S, D, H, HD = 2, 2048, 1024, 16, 64
P = 128
FD = 512          # fp32 psum bank / matmul moving-operand chunk
QG = 1024         # attention q-group size (psum bank budget)
NDT = D // P      # 8 D-tiles
NCH = S // FD     # 4 projection rhs chunks
NKT = S // P      # 16 k-tiles
NQG = S // QG     # 2
CW = NDT * FD     # x_sb columns per projection chunk
BN_EPS = 1e-12
NWARM = 8         # HAM-ramp dummy matmuls before the first projection

_PROG_CACHE = {}
LAST_EXEC_TIME_NS = None


def _install_ntff_hook():
    """This image's antenv package lacks axon_hooks; recreate it so
    run_bass_kernel_spmd(trace=True) can reach the NTFF profiler."""
    if "antenv.axon_hooks" in sys.modules:
        return
    if "/root/.axon_site" not in sys.path:
        sys.path.insert(0, "/root/.axon_site")
    try:
        from trn_agent_boot.trn_boot import _ntff_profile_via_ctypes
        hook = _ntff_profile_via_ctypes("/opt/axon/libaxon_pjrt.so")
    except Exception:
        hook = None
    m = types.ModuleType("antenv.axon_hooks")
    m.get_axon_ntff_profile_hook = lambda: hook
    m.set_axon_ntff_profile_hook = lambda h: None
    sys.modules["antenv.axon_hooks"] = m


def _dedupe_ldweights(nc, mybir):
    """The tile legalizer pre-splits 2-byte matmuls into LDWEIGHTS+MATMUL
    but emits one LDWEIGHTS per matmul even when consecutive matmuls share
    the stationary operand (and walrus's ldw-opt pass, which would fold
    them, rejects pre-split LDWEIGHTS).  Drop an LDWEIGHTS that reloads
    exactly what the PE already holds; a transpose matmul self-loads its
    identity, invalidating the tracked state."""
    for bb in nc.main_func.blocks:
        new = []
        last = None
        for ins in bb.instructions:
            if isinstance(ins, mybir.InstLdweights):
                a = ins.ins[0]
                sig = (a.memref, a.offset, tuple(map(tuple, a.ap)), a.dtype)
                si = ins.sync_info
                clean = si is None or (not si.on_wait and not si.on_update)
                if clean and sig == last:
                    continue
                last = sig
            elif isinstance(ins, mybir.InstMatmult):
                if getattr(ins, "is_transpose", False):
                    last = None
            new.append(ins)
        bb.instructions = new


def _split_sync_waits(nc, mybir):
    """This walrus build rejects instructions carrying more than one
    sync-wait command: hoist extra waits onto EventSemaphore
    instructions inserted just before (same engine stream, so the
    combined wait semantics are identical)."""
    for bb in nc.main_func.blocks:
        new = []
        for ins in bb.instructions:
            si = ins.sync_info
            if si is not None and si.on_wait and len(si.on_wait) > 1:
                waits = list(si.on_wait)
                for w in waits[:-1]:
                    new.append(mybir.InstEventSemaphore(
                        name=f"EVW-{nc.next_id()}",
                        engine=ins.engine,
                        ins=[], outs=[],
                        sync_info=mybir.SyncInfo(on_wait=[w], on_update=[]),
                    ))
                ins.sync_info = mybir.SyncInfo(
                    on_wait=[waits[-1]], on_update=list(si.on_update)
                )
            new.append(ins)
        bb.instructions = new


def _build(npair):
    import concourse.bass as bass
    import concourse.mybir as mybir
    import concourse.tile as tile

    f32 = mybir.dt.float32
    bf16 = mybir.dt.bfloat16
    ts = bass.ts
    _TC = tile.TileContext

    G = 3 * npair
    ns = 2 * npair
    nc = bass.Bass(num_devices=8)
    # xt arrives pre-swizzled by the host into the exact SBUF image
    # [P, NCH*CW] (chunk-major, 8KB contiguous per partition-row per
    # chunk) so each chunk is one large, descriptor-efficient DMA.
    xt = nc.dram_tensor("xt", [P, NCH * CW], bf16, kind="ExternalInput")
    wpk = nc.dram_tensor("wpk", [P, G * NDT * P], bf16, kind="ExternalInput")
    bpk = nc.dram_tensor("bpk", [P, G], f32, kind="ExternalInput")
    mk = nc.dram_tensor("mk", [P, NKT], f32, kind="ExternalInput")
    one = nc.dram_tensor("one", [P, NKT], bf16, kind="ExternalInput")
    # bf16 output: the host divides ctx^T by the rowsum anyway, so the
    # ~0.4% bf16 rounding of numerator and denominator costs ~2e-3 of
    # relative error (measured total stays under half the 2e-2 gate) and
    # halves the tail output-DMA bytes.
    out = nc.dram_tensor("out", [ns, 65, S], bf16, kind="ExternalOutput")
    fence = nc.dram_tensor("fence", [2, 4], bf16, kind="ExternalOutput")

    Exp = mybir.ActivationFunctionType.Exp

    with _TC(nc) as tc, \
         tc.tile_pool(name="const", bufs=1) as cpool, \
         tc.tile_pool(name="xtp", bufs=1) as xpool, \
         tc.tile_pool(name="qkv", bufs=npair) as qkvpool, \
         tc.tile_pool(name="vp", bufs=2) as vpool, \
         tc.tile_pool(name="ep", bufs=44) as epool, \
         tc.tile_pool(name="cup", bufs=4) as cupool:

        # Preload the ACT exp table while input DMAs run.
        warm = cpool.tile([P, 1], f32, name="warm", tag="warm")
        nc.vector.memset(warm[:], 0.0)
        warm2 = cpool.tile([P, 1], f32, name="warm2", tag="warm2")
        nc.scalar.activation(warm2[:], warm[:], Exp, bias=warm[:, 0:1])

        # HAM-ramp spam: dense full-array matmuls on a zeroed scratch
        # tile, no input dependencies, so the PE activity monitor lifts
        # the 1.2 GHz throttle during the DMA dead time instead of 10us
        # into the real projections.  The psum results are never read.
        wu_sb = cpool.tile([P, FD], bf16, name="wu", tag="wu")
        nc.vector.memset(wu_sb[:], 0.0)
        def mk_spam(pool):
            def spam(cols=FD):
                wps = pool.tile([P, FD], f32, name="wps", tag="wps")
                nc.tensor.matmul(wps[:, 0:cols], wu_sb[:, 0:P],
                                 wu_sb[:, 0:cols], start=True, stop=True)
            return spam

        wu_ctx = tc.tile_pool(name="wup", bufs=2, space="PSUM")
        spam0 = mk_spam(wu_ctx.__enter__())
        for _ in range(NWARM):
            spam0()
        wu_ctx.__exit__(None, None, None)

        # wpk is laid out K|Q|V-major by the host (all K groups first),
        # so the K-projection weights can be a small leading DMA and the
        # first matmul starts as early as possible.  x chunk quarters
        # alternate between the two HWDGE queues (sync/scalar); each DMA
        # op lands on its own SDMA engine set, so concurrency across ops
        # is what buys bandwidth.  ch0/ch1 are prioritized: the attention
        # loop needs only K-ch0 + Q-ch0/ch1 to start.
        WG = NDT * P                       # w columns per (type, pair) group
        w_sb = cpool.tile([P, G * WG], bf16, name="w", tag="w")
        x_sb = xpool.tile([P, NCH * CW], bf16, name="x", tag="x")
        QT4 = CW // 4
        # Phase 1: only what the attention loop needs to START (K+Q
        # weights, x-ch0, x-ch1).  The HWDGE queues run several ops
        # concurrently sharing bandwidth, so without this fence every
        # chunk completes near the END of the whole input stream
        # (~23us); narrowing the early queue gets ch0/ch1 in by ~14us.
        nc.sync.dma_start(w_sb[:, 0:npair * WG], wpk[:, 0:npair * WG])       # K
        for ch in range(2):
            for qq in range(4):
                eng = nc.scalar if qq % 2 == 0 else nc.sync
                eng.dma_start(
                    x_sb[:, ch * CW + qq * QT4: ch * CW + (qq + 1) * QT4],
                    xt[:, ch * CW + qq * QT4: ch * CW + (qq + 1) * QT4])
            if ch == 0:                                                       # Q
                nc.scalar.dma_start(
                    w_sb[:, npair * WG:2 * npair * WG],
                    wpk[:, npair * WG:2 * npair * WG])
        # Fences: a tiny readback of the tail of each queue's last ch1
        # piece; the phase-2 ops behind it wait for its semaphore, so
        # they stay out of the queue until ch1 has landed.  The fence
        # target (a corner of `out`) is overwritten by the real output.
        nc.sync.dma_start(fence[0:1, 0:1],
                          x_sb[0:1, 2 * CW - 4:2 * CW - 3])
        nc.scalar.dma_start(fence[1:2, 0:1],
                            x_sb[0:1, 2 * CW - QT4 - 1:2 * CW - QT4])
        for ch in range(2, NCH):
            dst = x_sb[:, ch * CW:(ch + 1) * CW]
            src = xt[:, ch * CW:(ch + 1) * CW]
            nc.sync.dma_start(dst[:, 0:CW // 2], src[:, 0:CW // 2])
            nc.scalar.dma_start(dst[:, CW // 2:CW], src[:, CW // 2:CW])
        nc.sync.dma_start(                                                    # V
            w_sb[:, 2 * npair * WG:3 * npair * WG],
            wpk[:, 2 * npair * WG:3 * npair * WG])
        b_sb = cpool.tile([P, G], f32, name="b", tag="b")
        nc.gpsimd.dma_start(b_sb[:], bpk[:, :])
        m_sb = cpool.tile([P, NKT], f32, name="m", tag="m")
        nc.gpsimd.dma_start(m_sb[:], mk[:, :])
        on_sb = cpool.tile([P, NKT], bf16, name="on", tag="on")
        nc.gpsimd.dma_start(on_sb[:], one[:, :])

        for p_ in range(npair):
            # Attention matmuls are deliberately FULL-ARRAY (K=128, M=128):
            # partial-array matmuls (K=64 scores / M=65 ctx) never register
            # as "busy" with the PE HAM activity monitor.  Q is stored
            # twice, zero-padded on the other slot's 64 partitions, so each
            # slot's scores matmul contracts over all 128 partitions against
            # the SHARED packed K stationary.
            kt_sb = qkvpool.tile([P, S], bf16, name="qkvK", tag="qkvK")
            qtz = [qkvpool.tile([P, S], bf16, name=f"qtz{hs}", tag=f"qtz{hs}")
                   for hs in range(2)]
            nc.vector.memset(qtz[0][HD:P, :], 0.0)
            nc.vector.memset(qtz[1][0:HD, :], 0.0)
            vps = []
            for hs in range(2):
                vp = vpool.tile([P, NKT * P], bf16, name="vp", tag="vp")
                nc.vector.memset(vp[:], 0.0)
                nc.vector.tensor_copy(
                    vp[:].rearrange("p (t c) -> p t c", c=P)[:, :, 64:65],
                    on_sb[:, 0:NKT].rearrange("p (t c) -> p t c", c=1),
                )
                vps.append(vp)

            gK = 0 * npair + p_
            gQ = 1 * npair + p_
            gV = 2 * npair + p_

            # PSUM budget: pp(2) + scs(2x2) = 6 banks while the deferred
            # projections overlap attention; pp closes once the last
            # projection quantum ran, freeing banks for the 4-bank ctx
            # accumulators (scs 4 + acc 4 = 8).
            ps_ctx = tc.tile_pool(name="ps", bufs=2, space="PSUM")
            pp_ctx = tc.tile_pool(name="pp", bufs=2, space="PSUM")
            pspool = ps_ctx.__enter__()
            pppool = pp_ctx.__enter__()
            vt_sb = qkvpool.tile([P, S], bf16, name="vtsb", tag="vtsb")
            tt_pool = tc.tile_pool(name="ttp", bufs=2)
            ttpool = tt_pool.__enter__()

            def proj_mms(g, ps, ch, d0, d1):
                for dt in range(d0, d1):
                    nc.tensor.matmul(
                        ps[:],
                        w_sb[:, (g * NDT + dt) * P:(g * NDT + dt + 1) * P],
                        x_sb[:, ch * CW + dt * FD: ch * CW + (dt + 1) * FD],
                        start=(dt == 0),
                        stop=(dt == NDT - 1),
                    )

            def k_finish(ch, ps):
                nc.vector.tensor_scalar_add(
                    kt_sb[:, ch * FD:(ch + 1) * FD], ps[:], b_sb[:, gK:gK + 1])

            def q_finish(ch, ps):
                nc.vector.tensor_scalar_add(
                    qtz[0][0:HD, ch * FD:(ch + 1) * FD], ps[0:HD, :],
                    b_sb[0:HD, gQ:gQ + 1])
                nc.vector.tensor_scalar_add(
                    qtz[1][HD:P, ch * FD:(ch + 1) * FD], ps[HD:P, :],
                    b_sb[HD:P, gQ:gQ + 1])

            # Minimal pre-loop projections: exactly what (qg0, kt0..3)
            # needs -- K-ch0 and Q-ch0/ch1.  The x quarters trickle in
            # ~2us apart while a projection piece takes ~0.9us, so small
            # spam matmuls are interleaved between the dt-pair pieces:
            # an idle PE of even ~1us makes the HAM re-throttle the
            # clock to 1.2 GHz for several us (observed), which is far
            # more expensive than the spam.
            wu2_ctx = tc.tile_pool(name="wup2", bufs=2, space="PSUM")
            spam = mk_spam(wu2_ctx.__enter__())
            for g, ch, fin in ((gK, 0, k_finish), (gQ, 0, q_finish),
                               (gQ, 1, q_finish)):
                ps = pppool.tile([P, FD], f32, name="pp", tag="pp")
                for dp in range(4):
                    proj_mms(g, ps, ch, dp * 2, dp * 2 + 2)
                    if dp % 2 == 1:
                        spam(256)
                fin(ch, ps)
            wu2_ctx.__exit__(None, None, None)

            def v_finish(ch, ps):
                # V chunk PSUM -> bf16 SBUF; the transposes that carve it
                # into V+ tiles are deferred until the x DMA has drained
                # (they share the HWDGE queues with the x input).
                nc.vector.tensor_copy(vt_sb[:, ch * FD:(ch + 1) * FD], ps[:])

            def t_tile(t):
                # One k-tile of V through the DMA xbar transpose engine
                # (zero PE cost) into a staging tile; two DVE copies split
                # the halves into the V+ tiles (col 64 of each V+ tile is
                # the preset ones column).
                def go():
                    vtt = ttpool.tile([P, P], bf16, name="vtt", tag="vtt")
                    eng = nc.sync if t % 2 == 0 else nc.scalar
                    eng.dma_start_transpose(vtt[:], vt_sb[:, ts(t, P)])
                    nc.vector.tensor_copy(
                        vps[0][:, t * P: t * P + HD], vtt[:, 0:HD])
                    nc.vector.tensor_copy(
                        vps[1][:, t * P: t * P + HD], vtt[:, HD:P])
                return go

            # Deferred projections become a queue of small PE quanta
            # (cost in us, earliest-step gate, must-pop flag) that fill
            # the PE's slack inside the attention loop below.  The
            # earliest-step gates respect x-chunk DMA arrival: the PE
            # queue executes in order, so a projection emitted before its
            # chunk has landed blocks every later PE instruction behind
            # it.  A must-pop item is emitted at its gate step even if
            # the budget ran out -- the next step's scores depend on it,
            # and emitting it any later would deadlock the PE queue.
            pstate = {}

            def p_mm(g, ch, d0, d1, fin):
                def go():
                    if d0 == 0:
                        pstate[(g, ch)] = pppool.tile(
                            [P, FD], f32, name="pp", tag="pp")
                    proj_mms(g, pstate[(g, ch)], ch, d0, d1)
                    if d1 == NDT:
                        fin(ch, pstate[(g, ch)])
                return go

            vwork = []
            for (g, ch, fin), mi, must in (
                ((gK, 1, k_finish), 0, True),   # before scores(kt4) @ 3
                ((gV, 0, v_finish), 1, False),  # x-ch0 resident
                ((gK, 2, k_finish), 3, True),   # before scores(kt8) @ 7
                ((gV, 1, v_finish), 4, False),
                ((gK, 3, k_finish), 5, True),   # before scores(kt12) @ 11
                ((gV, 2, v_finish), 6, False),
                ((gQ, 2, q_finish), 7, True),   # before scores(qg1) @ 15
                ((gQ, 3, q_finish), 8, True),
                ((gV, 3, v_finish), 9, False),
            ):
                vwork.append((p_mm(g, ch, 0, 4, fin), 0.9, mi, must))
                vwork.append((p_mm(g, ch, 4, NDT, fin), 0.9, mi, must))
            # V+ transposes: gated past the x DMA (done ~step 2-3 of the
            # loop) and their source chunk's projection.
            for t in range(NKT):
                gate = max(11, {0: 2, 1: 5, 2: 7, 3: 10}[t // 4] + 1)
                vwork.append((t_tile(t), 0.35, gate, False))
            vwork.sort(key=lambda w: w[2])

            def issue_scores(qg, kt, h2s):
                scs = [pspool.tile([P, QG], f32, name="ps", tag="ps")
                       for _ in range(2)]
                for hs in range(2):
                    for h2 in h2s:
                        nc.tensor.matmul(
                            scs[hs][:, h2 * FD:(h2 + 1) * FD],
                            kt_sb[:, ts(kt, P)],
                            qtz[hs][:, qg * QG + h2 * FD: qg * QG + (h2 + 1) * FD],
                            start=True, stop=True,
                        )
                return scs

            # ---- attention loop, software-pipelined --------------------
            # Step granularity is (qg, kt, h2-set).  The first four
            # k-tiles of qg0 are split into 512-column halves so the exp
            # stream starts after only x-ch0 + K-ch0 + Q-ch0 (h2=1 needs
            # Q-ch1, whose x chunk is still in flight); the last step is
            # split so the final ctx/out work overlaps the last exps.
            # ctx matmuls are deferred (exp outputs buffer in SBUF) until
            # the projection/V quanta finish and the psum banks can be
            # handed to the ctx accumulators.
            acc_ctx = [None]
            accpool_ref = [None]
            ctx_backlog = []          # (qg, kt, hs, h2, e-tile)
            ctx_accs = [None]
            cur_qg = [None]
            done15 = {}

            def open_acc_pool():
                pp_ctx.__exit__(None, None, None)
                acc_ctx[0] = tc.tile_pool(name="accp", bufs=2, space="PSUM")
                accpool_ref[0] = acc_ctx[0].__enter__()

            def drain_ctx(max_entries):
                done = 0
                while ctx_backlog and done < max_entries:
                    qg, kt, hs, h2, e = ctx_backlog.pop(0)
                    if cur_qg[0] != qg:
                        cur_qg[0] = qg
                        ctx_accs[0] = [
                            accpool_ref[0].tile([P, QG], f32, name="acc", tag="acc")
                            for _ in range(2)]
                    accs = ctx_accs[0]
                    nc.tensor.matmul(
                        accs[hs][:, h2 * FD:(h2 + 1) * FD],
                        vps[hs][:, kt * P:(kt + 1) * P],
                        e[:, h2 * FD:(h2 + 1) * FD],
                        start=(kt == 0),
                        stop=(kt == NKT - 1),
                    )
                    if kt == NKT - 1:
                        # bounce this h2-half of [ctx^T; rowsum] PSUM ->
                        # SBUF (hs0 on VectorE, hs1 on ScalarE -- idle
                        # after its last exp) and DMA it out split across
                        # both HWDGE queues, so the final output transfer
                        # overlaps the remaining halves' exps and ctx.
                        s_idx = p_ * 2 + hs
                        cu = cupool.tile([65, FD], bf16, name="cu", tag="cu")
                        cols = slice(h2 * FD, (h2 + 1) * FD)
                        nc.vector.tensor_copy(cu[:], accs[hs][0:65, cols])
                        o0 = qg * QG + h2 * FD
                        hf = FD // 2
                        nc.sync.dma_start(
                            out[s_idx][:, o0:o0 + hf], cu[:, 0:hf])
                        nc.scalar.dma_start(
                            out[s_idx][:, o0 + hf:o0 + FD], cu[:, hf:FD])
                    done += 1

            steps = []
            for kt in range(NKT):
                steps.append((0, kt, (0, 1)))
            for kt in range(NKT - 1):
                steps.append((1, kt, (0, 1)))
            steps.append((1, NKT - 1, (0,)))
            steps.append((1, NKT - 1, (1,)))

            cur = issue_scores(*steps[0])
            for i, (qg, kt, h2s) in enumerate(steps):
                c0, c1 = h2s[0] * FD, (h2s[-1] + 1) * FD
                es2 = []
                for hs in range(2):
                    e = epool.tile([P, QG], bf16, name="e", tag="e")
                    nc.scalar.activation(
                        e[:, c0:c1], cur[hs][:, c0:c1], Exp,
                        bias=m_sb[:, kt:kt + 1], scale=0.125,
                    )
                    es2.append(e)
                # next step's scores go on the PE queue FIRST so the scs
                # psum buffer refills the moment its exp frees it, keeping
                # ScalarE back-to-back.
                nxt = issue_scores(*steps[i + 1]) if i + 1 < len(steps) else None
                for hs in range(2):
                    for h2 in h2s:
                        ctx_backlog.append((qg, kt, hs, h2, es2[hs]))
                if vwork:
                    budget = 1.2
                    while vwork and vwork[0][2] <= i and \
                            (budget > 0 or vwork[0][3]):
                        go, cost, _mi, _must = vwork.pop(0)
                        go()
                        budget -= cost
                else:
                    if acc_ctx[0] is None:
                        open_acc_pool()
                    drain_ctx(6 if len(ctx_backlog) > 8 else 2)
                cur = nxt
            if acc_ctx[0] is None:
                open_acc_pool()
            drain_ctx(len(ctx_backlog))
            acc_ctx[0].__exit__(None, None, None)
            ps_ctx.__exit__(None, None, None)
            tt_pool.__exit__(None, None, None)
    _dedupe_ldweights(nc, mybir)
    _split_sync_waits(nc, mybir)
    return nc


def _np_gates(inputs):
    hs = inputs["hidden_states"].astype(np.float64)
    pooled = hs.mean(axis=1)
    h = pooled @ inputs["pW1"].astype(np.float64) + inputs["pb1"].astype(np.float64)
    h = (h - inputs["bn_mean"].astype(np.float64)) \
        / np.sqrt(inputs["bn_var"].astype(np.float64) + BN_EPS) \
        * inputs["bn_gamma"].astype(np.float64) + inputs["bn_beta"].astype(np.float64)
    h = np.maximum(h, 0.0)
    logits = h @ inputs["pW2"].astype(np.float64) + inputs["pb2"].astype(np.float64)
    return logits >= 0.0


def kernel(**inputs):
    global LAST_EXEC_TIME_NS
    import ml_dtypes
    bf = ml_dtypes.bfloat16

    inputs = {k: np.asarray(v) for k, v in inputs.items()}
    out_full = np.zeros((B, S, D), np.float32)

    gate = _np_gates(inputs)                       # [B, H] bool
    on = [[h for h in range(H) if gate[b, h]] for b in range(B)]
    n0, n1 = len(on[0]), len(on[1])
    if n0 + n1 == 0:
        return out_full

    # Split the 8 cores between the two batches to minimize the max
    # number of head-slots any core has to process.
    best = None
    for k0 in range(9):
        k1 = 8 - k0
        if (n0 > 0 and k0 == 0) or (n1 > 0 and k1 == 0):
            continue
        ns_req = max(
            math.ceil(n0 / k0) if n0 else 0,
            math.ceil(n1 / k1) if n1 else 0,
        )
        if best is None or ns_req < best[0]:
            best = (ns_req, k0)
    ns_req, k0 = best
    k1 = 8 - k0
    npair = (ns_req + 1) // 2
    ns = 2 * npair

    # head-slot assignment per core: (b, h, is_real)
    core_batch = [0 if c < k0 else 1 for c in range(8)]
    core_slots = []
    for c in range(8):
        b = core_batch[c]
        if b == 0:
            mine = on[0][c::k0] if k0 else []
        else:
            mine = on[1][(c - k0)::k1] if k1 else []
        slots = [(b, h, True) for h in mine]
        pad_h = mine[0] if mine else (on[b][0] if on[b] else 0)
        while len(slots) < ns:
            slots.append((b, pad_h, False))
        core_slots.append(slots)

    # per-batch staged arrays; x is pre-swizzled into the SBUF image
    # [P, NCH*CW]: row p, col ch*CW + dt*FD + j  <-  x^T[dt*P + p, ch*FD + j]
    xtb = []
    for b in range(B):
        xT = inputs["hidden_states"][b].T.astype(np.float32).astype(bf)  # [D, S]
        img = (xT.reshape(NDT, P, NCH, FD)      # (dt, p, ch, j)
               .transpose(1, 2, 0, 3)           # (p, ch, dt, j)
               .reshape(P, NCH * CW))
        xtb.append(np.ascontiguousarray(img))
    mkb = [np.ascontiguousarray(
        inputs["attention_mask"][b, 0, 0, :].astype(np.float32)
        .reshape(NKT, P).T) for b in range(B)]
    ones16 = np.ones((P, NKT), bf)

    Ws = (inputs["Wq"].astype(np.float32), inputs["Wk"].astype(np.float32),
          inputs["Wv"].astype(np.float32))
    bs = (inputs["bq"].astype(np.float32), inputs["bk"].astype(np.float32),
          inputs["bv"].astype(np.float32))

    G = 3 * npair
    in_maps = []
    for c in range(8):
        slots = core_slots[c]
        wgs, bgs = [], []
        # group order is type-major (all K pairs, then Q, then V) so the
        # K weights can be the first, small leading DMA on-device.
        for Wsrc, bsrc in ((Ws[1], bs[1]), (Ws[0], bs[0]), (Ws[2], bs[2])):
            for p_ in range(npair):
                h0 = slots[2 * p_][1]
                h1 = slots[2 * p_ + 1][1]
                wgs.append(np.concatenate(
                    [Wsrc[:, h0 * HD:(h0 + 1) * HD],
                     Wsrc[:, h1 * HD:(h1 + 1) * HD]], axis=1))
                bgs.append(np.concatenate(
                    [bsrc[h0 * HD:(h0 + 1) * HD],
                     bsrc[h1 * HD:(h1 + 1) * HD]]))
        wpk = (np.stack(wgs).reshape(G, NDT, P, P)
               .transpose(2, 0, 1, 3).reshape(P, G * NDT * P))
        bpk = np.stack(bgs, axis=1)
        b = core_batch[c]
        in_maps.append({
            "xt": xtb[b],
            "wpk": np.ascontiguousarray(wpk.astype(bf)),
            "bpk": np.ascontiguousarray(bpk),
            "mk": mkb[b],
            "one": ones16,
        })

    trace = os.environ.get("BASS_KERNEL_TRACE") == "1"
    if trace:
        _install_ntff_hook()

    # NOTE: --enable-ldw-opt stays false: the tile legalizer pre-splits
    # bf16 matmuls into LDWEIGHTS+MATMUL, which that walrus pass rejects.
    nc = _PROG_CACHE.get(npair)
    if nc is None:
        nc = _build(npair)
        _PROG_CACHE[npair] = nc

    from concourse.bass_utils import run_bass_kernel_spmd
    res = run_bass_kernel_spmd(
        nc, in_maps, core_ids=list(range(8)), trace=trace)
    LAST_EXEC_TIME_NS = res.exec_time_ns

    bv = inputs["bv"].astype(np.float32)
    for c in range(8):
        co = res.results[c]["out"]            # [ns, 65, S] bf16
        for si, (b, h, real) in enumerate(core_slots[c]):
            if real:
                blk = np.asarray(co[si]).astype(np.float32)
                out_full[b][:, h * HD:(h + 1) * HD] = \
                    (blk[0:64] / blk[64:65]).T + bv[h * HD:(h + 1) * HD][None, :]
    return out_full
